# revision 7
# baseline (speedup 1.0000x reference)
"""GraphSAGE-style 3-layer GNN (mean aggregation) on 8 Trainium2 NeuronCores.

Strategy (dst-sharded graph parallelism):
- Nodes (and their incoming edges) are sharded across 8 cores: core d owns
  dst nodes [d*6250, (d+1)*6250).
- Host preprocessing sorts each core's edges by dst node-tile (128 dst nodes
  per tile) and packs them into fixed 128-edge chunks on a uniform
  [49 tiles x M chunks] grid (padded with weight-0 edges).
- Aggregation on device: for each chunk, a one-hot matrix
  OH[e, n] = (dstloc[e] == n) * (1/deg) is built on the DVE from an iota tile,
  and  meanT[c, n] += G[e, c].T @ OH[e, n]  accumulates in PSUM on the PE.
- Layer-1 edge source features are pre-gathered on the host (edge-parallel
  input sharding) and streamed sequentially; layers 2/3 gather their source
  features from a replicated DRAM table via indirect DMA (128 rows/call).
- The replicated table is refreshed between layers with an AllGather
  (halo exchange) over the 8 cores.
- Dense transforms run in transposed layout: hT = Wl.T @ meanT + Wr.T @ xT,
  bias add + ReLU on the DVE, then PE-transposes produce row-major h for the
  next layer's gather table.
Compute dtype: bf16 inputs with fp32 PSUM accumulation.
"""
import numpy as np
import ml_dtypes

import concourse.bass as bass
import concourse.bacc as bacc
import concourse.mybir as mybir
import concourse.tile as tile
from concourse.bass_utils import run_bass_kernel_spmd

N = 50000
E = 800000
C1 = 128
HID = 256
OUT = 15
NCORES = 8
NP = N // NCORES          # 6250 own nodes per core
P = 128
TILES = (NP + P - 1) // P  # 49 node tiles per core
NPAD = TILES * P           # 6272
KB = 6                     # L1 staged-gather chunks per DMA

bf16 = mybir.dt.bfloat16
f32 = mybir.dt.float32
_bf = ml_dtypes.bfloat16


def _preprocess(x, edge_index):
    """Sort/pad edges into the uniform [NCORES, TILES, M, 128] chunk grid and
    build all per-core staged arrays."""
    src = np.ascontiguousarray(edge_index[0]).astype(np.int64)
    dst = np.ascontiguousarray(edge_index[1]).astype(np.int64)
    deg = np.bincount(dst, minlength=N)
    wnode = (1.0 / np.maximum(deg, 1)).astype(np.float32)

    core = dst // NP
    tilei = (dst - core * NP) // P
    gkey = core * TILES + tilei
    order = np.argsort(gkey, kind="stable")
    sg = gkey[order]
    ssrc = src[order]
    sdst = dst[order]
    starts = np.searchsorted(sg, np.arange(NCORES * TILES))
    counts = np.diff(np.append(starts, E))
    M = int((counts.max() + P - 1) // P)
    NCH = TILES * M

    # slot grids [NCORES*TILES, M*128]
    rank = np.arange(E) - starts[sg]
    slot = sg * (M * P) + rank
    src_grid = np.zeros(NCORES * TILES * M * P, np.int64)
    dstloc_grid = np.full(NCORES * TILES * M * P, P - 1, np.int64)
    w_grid = np.zeros(NCORES * TILES * M * P, np.float32)
    src_grid[slot] = ssrc
    dstloc_grid[slot] = sdst - (sdst // NP) * NP - ((sdst - (sdst // NP) * NP) // P) * P
    w_grid[slot] = wnode[sdst]

    src_grid = src_grid.reshape(NCORES, NCH, P)
    dstloc_grid = dstloc_grid.reshape(NCORES, NCH, P)
    w_grid = w_grid.reshape(NCORES, NCH, P)

    x_bf = x.astype(_bf)
    per_core = []
    for d in range(NCORES):
        idx32 = np.ascontiguousarray(src_grid[d].T).astype(np.int32)   # [128, NCH]
        dstw = np.empty((P, 2 * NCH), np.float32)
        dstw[:, 0::2] = dstloc_grid[d].T
        dstw[:, 1::2] = w_grid[d].T
        dstw = dstw.astype(_bf)
        # L1 pre-gathered edge features [128, NCH, C1]
        xg1 = np.ascontiguousarray(x_bf[src_grid[d]].transpose(1, 0, 2))
        # own transposed features [128, NPAD]
        xT = np.zeros((C1, NPAD), np.float32)
        xT[:, :NP] = x[d * NP:(d + 1) * NP].T
        per_core.append(dict(idx32=idx32, dstw=dstw, xg1=xg1,
                             xT=np.ascontiguousarray(xT).astype(_bf)))
    return per_core, M


def _build(nc: bass.Bass, M: int):
    NCH = TILES * M
    # ---- I/O ----
    idx32_d = nc.dram_tensor("idx32", [P, NCH], mybir.dt.int32, kind="ExternalInput")
    dstw_d = nc.dram_tensor("dstw", [P, 2 * NCH], bf16, kind="ExternalInput")
    xg1_d = nc.dram_tensor("xg1", [P, NCH, C1], bf16, kind="ExternalInput")
    xT_d = nc.dram_tensor("xT", [P, NPAD], bf16, kind="ExternalInput")
    iota_d = nc.dram_tensor("iota", [P, P], bf16, kind="ExternalInput")
    wl_d = {}
    wr_d = {}
    for l, cin in ((1, C1), (2, HID), (3, HID)):
        wl_d[l] = nc.dram_tensor(f"Wl{l}", [cin, HID], bf16, kind="ExternalInput")
        wr_d[l] = nc.dram_tensor(f"Wr{l}", [cin, HID], bf16, kind="ExternalInput")
    bl_d = nc.dram_tensor("bl", [P, 6], f32, kind="ExternalInput")       # [:, 2(l-1)+j]
    wo_d = nc.dram_tensor("Wo", [HID, OUT], bf16, kind="ExternalInput")
    bo_d = nc.dram_tensor("bo", [1, OUT], f32, kind="ExternalInput")
    out_d = nc.dram_tensor("out", [NP, OUT], f32, kind="ExternalOutput")

    KH = HID // P  # 2 halves of the hidden dim
    nblocks = [(b, min(512, NPAD - b)) for b in range(0, NPAD, 512)]

    with tile.TileContext(nc) as tc:
        with (
            tc.tile_pool(name="const", bufs=1) as cst,
            tc.tile_pool(name="feat", bufs=1) as featp,
            tc.tile_pool(name="g", bufs=8) as gp,
            tc.tile_pool(name="oh", bufs=2) as ohp,
            tc.tile_pool(name="stage", bufs=3) as stp,
            tc.tile_pool(name="pmean", bufs=2, space="PSUM") as pmean,
            tc.tile_pool(name="pdense", bufs=2, space="PSUM") as pdense,
            tc.tile_pool(name="ptr", bufs=1, space="PSUM") as ptr,
            tc.tile_pool(name="dram", bufs=1, space="DRAM") as dr,
        ):
            # ---- constants (funneled through DVE so consumers carry few waits)
            _cid = [0]
            def load_const(shape, dt, src_ap):
                _cid[0] += 1
                ld = cst.tile(shape, dt, tag=f"cld{_cid[0]}", name=f"cld{_cid[0]}")
                nc.sync.dma_start(ld[:], src_ap)
                t = cst.tile(shape, dt, tag=f"cst{_cid[0]}", name=f"cst{_cid[0]}")
                nc.vector.tensor_copy(t[:], ld[:])
                return t

            iota_sb = load_const([P, P], bf16, iota_d[:])
            dstw_sb = load_const([P, 2 * NCH], bf16, dstw_d[:])
            idx_sb = load_const([P, NCH], mybir.dt.int32, idx32_d[:])
            bl_sb = load_const([P, 6], f32, bl_d[:])
            wo_sb = [load_const([P, OUT], bf16, wo_d[h * P:(h + 1) * P, :])
                     for h in range(HID // P)]
            # bo broadcast to 128 partitions via DMA step-0
            bo_ld = cst.tile([P, OUT], f32)
            nc.sync.dma_start(bo_ld[:], bo_d[0:1, :].to_broadcast([P, OUT]))
            bo_sb = cst.tile([P, OUT], f32)
            nc.vector.tensor_copy(bo_sb[:], bo_ld[:])
            wl_sb = {}
            wr_sb = {}
            for l, cin in ((1, C1), (2, HID), (3, HID)):
                wl_sb[l] = [load_const([P, HID], bf16, wl_d[l][h * P:(h + 1) * P, :])
                            for h in range(cin // P)]
                wr_sb[l] = [load_const([P, HID], bf16, wr_d[l][h * P:(h + 1) * P, :])
                            for h in range(cin // P)]
            identity = cst.tile([P, P], bf16)
            from concourse.masks import make_identity
            make_identity(nc, identity[:])

            # ---- feature double buffers (transposed layout, [128, NPAD] per half)
            xT_sb = [featp.tile([P, NPAD], bf16, tag=f"ft0_{h}", name=f"xT_sb{h}") for h in range(KH)]
            hT_sb = [featp.tile([P, NPAD], bf16, tag=f"ft1_{h}", name=f"hT_sb{h}") for h in range(KH)]
            meanT_sb = [featp.tile([P, NPAD], bf16, tag=f"mt_{h}", name=f"meanT_sb{h}") for h in range(KH)]
            nc.sync.dma_start(xT_sb[0][:], xT_d[:])

            # DRAM halo buffers
            h_own = dr.tile([NP, HID], bf16)
            h_full = dr.tile([N, HID], bf16)

            def build_oh(t):
                """One-hot block for node-tile t: [128, M*128] bf16."""
                oh = ohp.tile([P, M * P], bf16, tag="oh")
                dloc = dstw_sb[:, 2 * t * M:2 * (t + 1) * M:2]          # [128, M]
                wcol = dstw_sb[:, 2 * t * M + 1:2 * (t + 1) * M:2]      # [128, M]
                nc.vector.tensor_tensor(
                    out=oh[:].rearrange("p (m n) -> p m n", m=M),
                    in0=dloc[:, :, None].to_broadcast([P, M, P]),
                    in1=iota_sb[:, None, :].to_broadcast([P, M, P]),
                    op=mybir.AluOpType.is_equal)
                nc.vector.tensor_tensor(
                    out=oh[:].rearrange("p (m n) -> p m n", m=M),
                    in0=oh[:].rearrange("p (m n) -> p m n", m=M),
                    in1=wcol[:, :, None].to_broadcast([P, M, P]),
                    op=mybir.AluOpType.mult)
                return oh

            def aggregate(layer, cin):
                """meanT_sb <- segment-mean of gathered source features."""
                khalves = cin // P
                for t in range(TILES):
                    oh = build_oh(t)
                    pm = [pmean.tile([P, P], f32, tag=f"pm{h}", space="PSUM",
                                     name=f"pm_{t}_{h}") for h in range(khalves)]
                    if layer == 1:
                        for mb_ in range(0, M, KB):
                            nb = min(KB, M - mb_)
                            g = gp.tile([P, KB * C1], bf16, tag="g1")
                            nc.sync.dma_start(
                                g[:, :nb * C1],
                                xg1_d[:, t * M + mb_:t * M + mb_ + nb, :])
                            for j in range(nb):
                                m = mb_ + j
                                nc.tensor.matmul(
                                    pm[0][:], lhsT=g[:, j * C1:(j + 1) * C1],
                                    rhs=oh[:, m * P:(m + 1) * P],
                                    start=(m == 0), stop=(m == M - 1))
                    else:
                        for m in range(M):
                            k = t * M + m
                            g = gp.tile([P, HID], bf16, tag="g2")
                            nc.gpsimd.indirect_dma_start(
                                out=g[:], out_offset=None, in_=h_full[:],
                                in_offset=bass.IndirectOffsetOnAxis(
                                    ap=idx_sb[:, k:k + 1], axis=0))
                            for h in range(khalves):
                                nc.tensor.matmul(
                                    pm[h][:], lhsT=g[:, h * P:(h + 1) * P],
                                    rhs=oh[:, m * P:(m + 1) * P],
                                    start=(m == 0), stop=(m == M - 1))
                    for h in range(khalves):
                        nc.vector.tensor_copy(
                            meanT_sb[h][:, t * P:(t + 1) * P], pm[h][:])

            def dense(layer, cin, src_feat, dst_feat):
                """dst_feat[j] = relu(Wl.T @ meanT + Wr.T @ src_feat + bl)."""
                khalves = cin // P
                for j in range(KH):
                    for b0, blen in nblocks:
                        pd = pdense.tile([P, 512], f32, tag="pd", space="PSUM")
                        nmm = 2 * khalves
                        i = 0
                        for h in range(khalves):
                            nc.tensor.matmul(
                                pd[:, :blen],
                                lhsT=wl_sb[layer][h][:, j * P:(j + 1) * P],
                                rhs=meanT_sb[h][:, b0:b0 + blen],
                                start=(i == 0), stop=(i == nmm - 1)); i += 1
                            nc.tensor.matmul(
                                pd[:, :blen],
                                lhsT=wr_sb[layer][h][:, j * P:(j + 1) * P],
                                rhs=src_feat[h][:, b0:b0 + blen],
                                start=(i == 0), stop=(i == nmm - 1)); i += 1
                        tmp = stp.tile([P, 512], f32, tag="dtmp")
                        nc.vector.tensor_tensor(
                            out=tmp[:, :blen], in0=pd[:, :blen],
                            in1=bl_sb[:, 2 * (layer - 1) + j:2 * (layer - 1) + j + 1]
                                .to_broadcast([P, blen]),
                            op=mybir.AluOpType.add)
                        nc.vector.tensor_scalar(
                            out=dst_feat[j][:, b0:b0 + blen], in0=tmp[:, :blen],
                            scalar1=0.0, scalar2=None,
                            op0=mybir.AluOpType.max)

            def write_rows(feat, last_layer):
                """Transpose hT -> row-major h_own, then AllGather into h_full."""
                for t in range(TILES):
                    rows = stp.tile([P, HID], bf16, tag="rows")
                    for j in range(KH):
                        pt = ptr.tile([P, P], bf16, tag="pt", space="PSUM")
                        nc.tensor.transpose(
                            pt[:], feat[j][:, t * P:(t + 1) * P], identity[:])
                        nc.vector.tensor_copy(rows[:, j * P:(j + 1) * P], pt[:])
                    nrow = min(P, NP - t * P)
                    nc.sync.dma_start(h_own[t * P:t * P + nrow, :], rows[:nrow, :])
                if not last_layer:
                    nc.gpsimd.collective_compute(
                        "AllGather", mybir.AluOpType.bypass,
                        replica_groups=[list(range(NCORES))],
                        ins=[h_own[:]], outs=[h_full[:]])

            # ---- layer 1
            aggregate(1, C1)
            dense(1, C1, xT_sb, hT_sb)
            write_rows(hT_sb, last_layer=False)
            # ---- layer 2
            aggregate(2, HID)
            dense(2, HID, hT_sb, xT_sb)   # ping-pong: xT_sb now holds h2T
            write_rows(xT_sb, last_layer=False)
            # ---- layer 3
            aggregate(3, HID)
            dense(3, HID, xT_sb, hT_sb)   # hT_sb now holds h3T
            # ---- output layer: out[n, :] = h3.T @ Wo + bo
            for t in range(TILES):
                po = ptr.tile([P, OUT], f32, tag="po", space="PSUM")
                for h in range(KH):
                    nc.tensor.matmul(
                        po[:], lhsT=hT_sb[h][:, t * P:(t + 1) * P],
                        rhs=wo_sb[h][:],
                        start=(h == 0), stop=(h == KH - 1))
                orow = stp.tile([P, OUT], f32, tag="orow")
                nc.vector.tensor_tensor(out=orow[:], in0=po[:], in1=bo_sb[:],
                                        op=mybir.AluOpType.add)
                nrow = min(P, NP - t * P)
                nc.sync.dma_start(out_d[t * P:t * P + nrow, :], orow[:nrow, :])
    return nc


_PROGRAM_CACHE = {}


def _get_program(M):
    if M not in _PROGRAM_CACHE:
        nc = bacc.Bacc("TRN2", target_bir_lowering=False, debug=False,
                       num_devices=NCORES)
        _build(nc, M)
        nc.compile()
        _PROGRAM_CACHE[M] = nc
    return _PROGRAM_CACHE[M]


def make_in_maps(inputs):
    x = np.asarray(inputs["x"], np.float32)
    per_core, M = _preprocess(x, np.asarray(inputs["edge_index"]))
    iota = np.tile(np.arange(P, dtype=np.float32)[None, :], (P, 1)).astype(_bf)
    bl = np.zeros((P, 6), np.float32)
    for l in (1, 2, 3):
        b = np.asarray(inputs[f"bl{l}"], np.float32)
        bl[:, 2 * (l - 1)] = b[:P]
        bl[:, 2 * (l - 1) + 1] = b[P:]
    common = {"iota": iota, "bl": bl,
              "Wo": np.asarray(inputs["Wo"]).astype(_bf),
              "bo": np.asarray(inputs["bo"], np.float32).reshape(1, OUT)}
    for l in (1, 2, 3):
        common[f"Wl{l}"] = np.asarray(inputs[f"Wl{l}"]).astype(_bf)
        common[f"Wr{l}"] = np.asarray(inputs[f"Wr{l}"]).astype(_bf)
    in_maps = []
    for d in range(NCORES):
        pc = per_core[d]
        in_maps.append({**common, "idx32": pc["idx32"], "dstw": pc["dstw"],
                        "xg1": pc["xg1"], "xT": pc["xT"]})
    return in_maps, M


def kernel(**inputs) -> np.ndarray:
    in_maps, M = make_in_maps(inputs)
    nc = _get_program(M)
    res = run_bass_kernel_spmd(nc, in_maps, core_ids=list(range(NCORES)))
    out = np.concatenate(
        [np.asarray(res.results[d]["out"], np.float32) for d in range(NCORES)], axis=0)
    return out
